# revision 12
# baseline (speedup 1.0000x reference)
"""Bayesian linear layer on 8 TRN2 NeuronCores.

Computes (reference semantics):
    x_aug   = concat([x, ones], axis=1)            # (B, IN+1)
    weights = q_mu + exp(q_log_sigma) * eps        # (OUT, IN+1)
    outputs = x_aug @ weights.T                    # (B, OUT)
    kl      = sum(p_ls - q_ls + (exp(q_ls)^2 + (p_mu-q_mu)^2)/(2 e^{2 p_ls}) - 0.5)

Sharding: tensor-parallel over OUT (512 out-features per core). Every core
streams the full transposed activations x^T once; weight-sample inputs,
p_mu and the KL shard are per-core column slices. The matmul runs in
float32r (~1 cycle/row on the PE for N>=256) with fp32 PSUM accumulation;
the bias column is excluded from the matmul and applied in fp32 during the
PSUM->SBUF eviction. KL partial sums come out as [128, 3] per core and are
combined on the host (the "scalar all-reduce").

Schedule: phase B is k-outer over groups of 1024 batch columns - 8 PSUM
chains (all banks) accumulate while x^T arrives as [128, 1024] k-slice
tiles whose rows are 4 KiB contiguous (measured ~2.3x the DMA bandwidth of
1 KiB chunks on TRN2). Group 0's x tiles are loaded inside the phase-A
emission so its chains consume weight k-tiles as they are produced,
hiding the weight-sample production behind matmul.
"""
import sys

sys.path.insert(0, "/opt/trn_rl_repo")

import math

import numpy as np
import ml_dtypes

import concourse.bacc as bacc
import concourse.bass as bass
import concourse.mybir as mybir
from concourse.tile import TileContext
from concourse.bass_utils import run_bass_kernel_spmd

B, IN, OUT = 8192, 4096, 4096
KP1 = IN + 1           # weight columns incl. bias
NCORES = 8
OSH = OUT // NCORES    # out-features per core (512)
NKT = IN // 128        # full 128-row k-tiles (32)
GC = 1024              # batch columns per group (8 psum chains of 128)
NG = B // GC           # 8 groups
F32 = mybir.dt.float32
F32R = mybir.dt.float32r
BF16 = mybir.dt.bfloat16
Exp = mybir.ActivationFunctionType.Exp
Square = mybir.ActivationFunctionType.Square
ADD = mybir.AluOpType.add
X_AXIS = mybir.AxisListType.X

_cache: dict = {}


def _build(repeats: int = 1):
    key = ("nc", repeats)
    if key in _cache:
        return _cache[key]
    nc = bacc.Bacc("TRN2", target_bir_lowering=False)
    xT = nc.dram_tensor("xT", [IN, B], F32R, kind="ExternalInput")
    qmT = nc.dram_tensor("qmT", [KP1, OSH], F32, kind="ExternalInput")
    qlT = nc.dram_tensor("qlT", [KP1, OSH], F32, kind="ExternalInput")
    epT = nc.dram_tensor("epT", [KP1, OSH], F32, kind="ExternalInput")
    pmT = nc.dram_tensor("pmT", [KP1, OSH], BF16, kind="ExternalInput")
    out = nc.dram_tensor("out", [B, OSH], F32, kind="ExternalOutput")
    klp = nc.dram_tensor("klp", [128, 3], F32, kind="ExternalOutput")

    with TileContext(nc) as tc:
      for _rep in range(repeats):
        with (
            tc.tile_pool(name="wres", bufs=1) as wres,
            tc.tile_pool(name="ins", bufs=2) as insp,
            tc.tile_pool(name="scr", bufs=2) as scr,
            tc.tile_pool(name="xp", bufs=10) as xp,
            tc.tile_pool(name="obp", bufs=4) as obp,
            tc.tile_pool(name="klq", bufs=1) as klq,
            tc.tile_pool(name="ps", bufs=8, space="PSUM") as psp,
        ):
            W = wres.tile([128, NKT, OSH], F32R, tag="W")        # resident weightsT
            wb = wres.tile([1, OSH], F32, tag="wb")              # bias row (k=4096)
            bias_bc = wres.tile([128, OSH], F32, tag="bias_bc")  # bias bcast to 128p
            s1c = klq.tile([128, NKT + 1], F32, tag="s1c")       # rowsums exp(2 q_ls)
            s2c = klq.tile([128, NKT + 1], F32, tag="s2c")       # rowsums (p_mu-q_mu)^2
            s3c = klq.tile([128, NKT + 1], F32, tag="s3c")       # rowsums q_ls
            nc.vector.memset(s1c, 0.0)
            nc.vector.memset(s2c, 0.0)
            nc.vector.memset(s3c, 0.0)

            def load_xg(g, kt):
                xg = xp.tile([128, GC], F32R, tag="xg", name=f"xg{g}_{kt}")
                nc.sync.dma_start(
                    out=xg,
                    in_=xT.ap()[kt * 128:(kt + 1) * 128, g * GC:(g + 1) * GC])
                return xg

            # ---- phase A: weight sample + KL partials, streamed by k-tile;
            # group 0's x tiles ride along so matmul can start early.
            # The first PRE tiles are loaded up front so the weight-input
            # stream (not x) paces the rest of phase A. ----
            PRE = 0
            g0_x = {}
            for kt in range(PRE):
                g0_x[kt] = load_xg(0, kt)
            for kt in range(NKT + 1):
                p = 128 if kt < NKT else 1
                r0 = kt * 128
                qm = insp.tile([128, OSH], F32, tag="qm")
                ql = insp.tile([128, OSH], F32, tag="ql")
                ep = insp.tile([128, OSH], F32, tag="ep")
                pm = insp.tile([128, OSH], BF16, tag="pm")
                nc.sync.dma_start(out=qm[:p], in_=qmT.ap()[r0:r0 + p, :])
                nc.sync.dma_start(out=ql[:p], in_=qlT.ap()[r0:r0 + p, :])
                nc.sync.dma_start(out=ep[:p], in_=epT.ap()[r0:r0 + p, :])
                nc.sync.dma_start(out=pm[:p], in_=pmT.ap()[r0:r0 + p, :])
                if kt + PRE < NKT:
                    g0_x[kt + PRE] = load_xg(0, kt + PRE)

                sig = scr.tile([128, OSH], F32, tag="sig")
                nc.scalar.activation(sig[:p], ql[:p], Exp)
                tmp = scr.tile([128, OSH], F32, tag="tmp")
                nc.vector.tensor_mul(out=tmp[:p], in0=sig[:p], in1=ep[:p])
                if kt < NKT:
                    nc.vector.tensor_add(out=W[:, kt, :], in0=qm, in1=tmp)
                else:
                    nc.vector.tensor_add(out=wb, in0=qm[:1], in1=tmp[:1])

                # KL partials for this core's o-slice
                sq = scr.tile([128, OSH], F32, tag="sq")
                nc.scalar.activation(sq[:p], ql[:p], Exp, scale=2.0,
                                     accum_out=s1c[:p, kt:kt + 1])
                d = scr.tile([128, OSH], F32, tag="d")
                nc.vector.tensor_sub(out=d[:p], in0=pm[:p], in1=qm[:p])
                sq2 = scr.tile([128, OSH], F32, tag="sq")
                nc.scalar.activation(sq2[:p], d[:p], Square,
                                     accum_out=s2c[:p, kt:kt + 1])
                nc.vector.tensor_reduce(out=s3c[:p, kt:kt + 1], in_=ql[:p],
                                        axis=X_AXIS, op=ADD)

            nc.gpsimd.partition_broadcast(bias_bc, wb)

            # ---- phase B: k-outer groups, 8 psum chains per group ----
            for g in range(NG):
                pss = [psp.tile([128, OSH], F32, tag="ps", name=f"ps{g}_{s}")
                       for s in range(GC // 128)]
                for kt in range(NKT):
                    xg = g0_x[kt] if g == 0 else load_xg(g, kt)
                    for s in range(GC // 128):
                        nc.tensor.matmul(pss[s],
                                         lhsT=xg[:, s * 128:(s + 1) * 128],
                                         rhs=W[:, kt, :],
                                         start=(kt == 0), stop=(kt == NKT - 1))
                for s in range(GC // 128):
                    ob = obp.tile([128, OSH], F32, tag="ob")
                    nc.vector.tensor_add(out=ob, in0=pss[s], in1=bias_bc)
                    m0 = g * GC + s * 128
                    nc.sync.dma_start(out=out.ap()[m0:m0 + 128, :], in_=ob)

            # ---- KL partial pack ----
            klout = klq.tile([128, 3], F32, tag="klout")
            nc.vector.tensor_reduce(out=klout[:, 0:1], in_=s1c, axis=X_AXIS, op=ADD)
            nc.vector.tensor_reduce(out=klout[:, 1:2], in_=s2c, axis=X_AXIS, op=ADD)
            nc.vector.tensor_reduce(out=klout[:, 2:3], in_=s3c, axis=X_AXIS, op=ADD)
            nc.sync.dma_start(out=klp.ap(), in_=klout)

    nc.finalize()
    _cache[key] = nc
    return nc


def kernel(x, p_log_sigma, q_mu, q_log_sigma, p_mu, eps):
    nc = _build()
    xT = np.ascontiguousarray(np.asarray(x).T)
    q_mu = np.asarray(q_mu)
    q_log_sigma = np.asarray(q_log_sigma)
    p_mu = np.asarray(p_mu)
    eps = np.asarray(eps)
    in_maps = []
    for c in range(NCORES):
        sl = slice(c * OSH, (c + 1) * OSH)
        in_maps.append({
            "xT": xT,
            "qmT": np.ascontiguousarray(q_mu[sl].T),
            "qlT": np.ascontiguousarray(q_log_sigma[sl].T),
            "epT": np.ascontiguousarray(eps[sl].T),
            "pmT": np.ascontiguousarray(p_mu[sl].T).astype(ml_dtypes.bfloat16),
        })
    res = run_bass_kernel_spmd(nc, in_maps, core_ids=list(range(NCORES)))
    outputs = np.concatenate([res.results[c]["out"] for c in range(NCORES)], axis=1)

    s = np.zeros(3, np.float64)
    for c in range(NCORES):
        s += res.results[c]["klp"].astype(np.float64).sum(axis=0)
    S1, S2, S3 = s
    pls = float(np.asarray(p_log_sigma))
    n_el = OUT * KP1
    kl = n_el * pls - S3 + 0.5 * math.exp(-2.0 * pls) * (S1 + S2) - 0.5 * n_el
    return outputs, np.float32(kl)
